# revision 10
# baseline (speedup 1.0000x reference)
"""Trainium2 Bass kernel for nn_ItemEncoder.

Computation:
    h_type = emb[item_type]                      # [bs, na, ni, 32]
    h = concat([h_type, item], -1)               # [bs, na, ni, 43]
    z = h @ W + b                                # [bs, na, ni, 128]
    out = max_{ni} relu(z)                       # [bs, na, 128]

Device strategy (pure data parallel over bs, 4 batches/core):
    Fold the embedding gather + bias into the matmul:
        T = emb @ W[:32] + b                     # (18, 128) tiny table
        z_tok = [x_tok ; onehot(t_tok)] @ [W2 ; T]   (K = 11 + 18 = 29)
    Host packs rhs [29, ntok] (features + one-hot), device runs K=29
    matmuls into PSUM and reduces max over ni=128 token groups on DVE
    (relu commutes with max, applied once at the end on the [128, 512]
    result).  Output is [h=128, group=512] per core; host transposes.
"""

import sys

sys.path.insert(0, "/opt/trn_rl_repo")

import ml_dtypes
import numpy as np

import concourse.bass as bass
import concourse.tile as tile
from concourse import bacc, mybir
from concourse import bass_utils

BS, NA, NI, F, H = 32, 128, 128, 11, 128
NTYPE, KEMB = 18, 32
NCORES = 8
BPC = BS // NCORES          # batches per core = 4
G = BPC * NA                # (b, na) groups per core = 512
TOK = G * NI                # tokens per core = 65536
K = F + NTYPE               # contraction dim = 29
CHUNK = 2048                # tokens per chunk (4 psum banks)
NCHUNK = TOK // CHUNK       # 32
F32 = mybir.dt.float32
BF16 = mybir.dt.bfloat16

_cache = {}


def _build_program(repeat=1):
    key = ("nc", repeat)
    if key in _cache:
        return _cache[key]

    nc = bacc.Bacc(
        "TRN2",
        target_bir_lowering=False,
        debug=False,
        enable_asserts=False,
        num_devices=NCORES,
    )

    rhs_d = nc.dram_tensor("rhs", [NCHUNK, K, CHUNK], BF16, kind="ExternalInput").ap()
    lhsT_d = nc.dram_tensor("lhsT", [K, H], BF16, kind="ExternalInput").ap()
    out_d = nc.dram_tensor("out", [H, G], F32, kind="ExternalOutput").ap()

    with tile.TileContext(nc) as tc:
        with (
            tc.tile_pool(name="const", bufs=1) as const_pool,
            tc.tile_pool(name="rhs", bufs=4) as rhs_pool,
            tc.tile_pool(name="ps", bufs=2, space=bass.MemorySpace.PSUM) as ps_pool,
            tc.tile_pool(name="res", bufs=1) as res_pool,
        ):
            lt = const_pool.tile([K, H], BF16)
            nc.sync.dma_start(lt[:], lhsT_d[:])

            def body():
                ob = res_pool.tile([H, G], F32)
                orelu = res_pool.tile([H, G], F32)

                for j in range(NCHUNK):
                    r = rhs_pool.tile([K, CHUNK], BF16)
                    nc.sync.dma_start(r[:], rhs_d[j])

                    p = ps_pool.tile([H, CHUNK], F32)
                    for k in range(CHUNK // 512):
                        nc.tensor.matmul(
                            p[:, k * 512:(k + 1) * 512],
                            lt[:],
                            r[:, k * 512:(k + 1) * 512],
                        )

                    # max over ni=128 within each (b, na) group
                    gpc = CHUNK // NI  # groups per chunk = 16
                    nc.vector.reduce_max(
                        ob[:, j * gpc:(j + 1) * gpc],
                        p[:].rearrange("p (g i) -> p g i", i=NI),
                        axis=mybir.AxisListType.X,
                    )

                nc.scalar.activation(
                    orelu[:], ob[:], mybir.ActivationFunctionType.Relu
                )
                nc.sync.dma_start(out_d[:], orelu[:])

            if repeat == 1:
                body()
            else:
                with tc.For_i(0, repeat, 1):
                    body()

    nc.compile()
    _cache[key] = nc
    return nc


def _pack_inputs(item_type, item, emb, W, b):
    T_tab = (emb.astype(np.float32) @ W[:KEMB].astype(np.float32)
             + b.astype(np.float32))                       # (18, 128)
    lhsT = np.concatenate(
        [W[KEMB:].astype(np.float32), T_tab], axis=0
    ).astype(ml_dtypes.bfloat16)                           # (29, 128)
    eye = np.eye(NTYPE, dtype=ml_dtypes.bfloat16)

    in_maps = []
    for c in range(NCORES):
        x = item[c * BPC:(c + 1) * BPC].astype(np.float32).reshape(TOK, F)
        t = np.asarray(item_type[c * BPC:(c + 1) * BPC]).reshape(TOK)
        rhs = np.empty((K, TOK), dtype=ml_dtypes.bfloat16)
        rhs[:F] = x.T.astype(ml_dtypes.bfloat16)
        rhs[F:] = eye[t].T                                  # one-hot rows
        rhs = np.ascontiguousarray(
            rhs.reshape(K, NCHUNK, CHUNK).transpose(1, 0, 2)
        )                                                   # (32, 29, 2048)
        in_maps.append({"rhs": rhs, "lhsT": lhsT})
    return in_maps


def _run(in_maps, trace=False, repeat=1):
    nc = _build_program(repeat)
    return bass_utils.run_bass_kernel_spmd(
        nc, in_maps, core_ids=list(range(NCORES)), trace=trace
    )


def kernel(item_type, item, emb, W, b):
    in_maps = _pack_inputs(item_type, item, emb, W, b)
    res = _run(in_maps, trace=False)
    out = np.empty((BS, NA, H), dtype=np.float32)
    for c in range(NCORES):
        o = res.results[c]["out"]                           # (128, 512) [h, g]
        out[c * BPC:(c + 1) * BPC] = o.T.reshape(BPC, NA, H)
    return out
